# revision 16
# baseline (speedup 1.0000x reference)
"""Two-layer GCN encoder (GCNConv -> BatchNorm -> ELU -> GCNConv) on 8 trn2
NeuronCores.

Sharding: nodes are partitioned across the 8 cores (graph/data parallel).
Each core computes h = x_local @ W1 (pre-scaled by deg^-1/2), the scaled
feature table is AllGathered, and each core aggregates messages for its own
destination nodes by gathering source rows with dma_gather and scatter-adding
them into PSUM via one-hot selection-matrix matmuls (128 edges per matmul).
BN statistics are AllReduced. Layer 2 aggregates the 128-wide hidden features
first and applies W2 afterwards ( A(hW) == (Ah)W ).

Self-loops are applied as an extra identity matmul per destination tile (the
local g rows are resident in SBUF), not as explicit edges.
"""

import numpy as np
import ml_dtypes

import concourse.bass as bass
import concourse.bacc as bacc
import concourse.mybir as mybir
import concourse.tile as tile
from concourse.alu_op_type import AluOpType
from concourse import library_config
from concourse.bass_utils import run_bass_kernel_spmd

P = 128
M = 8  # cores
BF16 = ml_dtypes.bfloat16
AF = mybir.ActivationFunctionType


# --------------------------------------------------------------------------
# Host-side preprocessing
# --------------------------------------------------------------------------

def preprocess(edge_index, n, gt_tiles=2):
    """Sort/partition edges, build per-core padded index arrays and the
    (uniform across cores) block structure.

    Returns (meta, per_core) where meta has the compile-time structure and
    per_core the per-core numpy arrays.
    """
    src = np.asarray(edge_index[0], dtype=np.int64)
    dst = np.asarray(edge_index[1], dtype=np.int64)
    e = src.shape[0]

    n_per = n // M
    assert n_per * M == n
    T = (n_per + P - 1) // P
    ncp = T * P                      # padded nodes per core
    R = M * ncp                      # rows in the gathered table
    Rh = R // 2                      # half split (must fit int16)
    assert Rh < 32768, f"half table {Rh} rows exceeds int16 range"
    assert (4 * ncp) == Rh           # half boundary aligns with core boundary

    deg = np.bincount(dst, minlength=n).astype(np.float64) + 1.0
    dinv = (deg ** -0.5).astype(np.float32)

    # padded global row of each source node
    owner = src // n_per
    r_src = owner * ncp + (src - owner * n_per)          # [e]
    half = (r_src >= Rh).astype(np.int64)
    idx16 = (r_src - half * Rh).astype(np.int64)

    core_of = dst // n_per
    l_dst = dst - core_of * n_per
    t_dst = l_dst // P
    dstloc = l_dst % P

    # per (core, tile, half) edge counts -> uniform block counts
    counts = np.zeros((M, T, 2), dtype=np.int64)
    np.add.at(counts, (core_of, t_dst, half), 1)
    Bth = np.ceil(counts.max(axis=0) / P).astype(np.int64)   # [T, 2]

    # groups of tiles for gathers
    groups = []
    off_lo = off_hi = 0
    for g0 in range(0, T, gt_tiles):
        tiles = list(range(g0, min(g0 + gt_tiles, T)))
        nb_lo = int(sum(Bth[t, 0] for t in tiles))
        nb_hi = int(sum(Bth[t, 1] for t in tiles))
        groups.append(dict(tiles=tiles, off_lo=off_lo, nb_lo=nb_lo,
                           off_hi=off_hi, nb_hi=nb_hi))
        off_lo += nb_lo
        off_hi += nb_hi
    NB_lo, NB_hi = off_lo, off_hi

    # per-tile block index lists (global block number within its half array)
    tile_blocks = []   # [T] -> (lo_block_ids, hi_block_ids)
    blo = bhi = 0
    for t in range(T):
        lo_ids = list(range(blo, blo + int(Bth[t, 0]))); blo += int(Bth[t, 0])
        hi_ids = list(range(bhi, bhi + int(Bth[t, 1]))); bhi += int(Bth[t, 1])
        tile_blocks.append((lo_ids, hi_ids))

    meta = dict(n=n, e=e, n_per=n_per, T=T, ncp=ncp, R=R, Rh=Rh,
                Bth=Bth, groups=groups, tile_blocks=tile_blocks,
                NB_lo=NB_lo, NB_hi=NB_hi)

    # --- per-core arrays ---
    order = np.lexsort((half, t_dst, core_of))   # sort by core, tile, half
    src_s = idx16[order]
    dl_s = dstloc[order]
    c_s = core_of[order]
    t_s = t_dst[order]
    h_s = half[order]

    per_core = []
    # compute start offset of each (c,t,h) run via counts prefix
    flat_counts = counts  # [M,T,2]
    run_start = np.zeros((M, T, 2), dtype=np.int64)
    np.cumsum(flat_counts.reshape(-1), out=run_start.reshape(-1))
    run_start = run_start - flat_counts  # exclusive prefix

    for c in range(M):
        idx_lo = np.zeros(max(NB_lo, 1) * P, dtype=np.int16)
        dl_lo = np.full(max(NB_lo, 1) * P, 255.0, dtype=np.float32)
        idx_hi = np.zeros(max(NB_hi, 1) * P, dtype=np.int16)
        dl_hi = np.full(max(NB_hi, 1) * P, 255.0, dtype=np.float32)
        for t in range(T):
            for h, (idx_arr, dl_arr, blk_ids) in enumerate(
                    ((idx_lo, dl_lo, tile_blocks[t][0]),
                     (idx_hi, dl_hi, tile_blocks[t][1]))):
                cnt = int(flat_counts[c, t, h])
                s0 = int(run_start[c, t, h])
                if not blk_ids:
                    assert cnt == 0
                    continue
                dst_off = blk_ids[0] * P
                idx_arr[dst_off:dst_off + cnt] = src_s[s0:s0 + cnt]
                dl_arr[dst_off:dst_off + cnt] = dl_s[s0:s0 + cnt]
                assert cnt <= len(blk_ids) * P

        def wrap16(a):
            w = a.reshape(-1, 16).T.copy()          # [16, L/16]
            return np.tile(w, (8, 1))               # [128, L/16]

        per_core.append(dict(
            idx_lo=wrap16(idx_lo), idx_hi=wrap16(idx_hi),
            dl_lo=dl_lo.reshape(-1, P).T.copy(),
            dl_hi=dl_hi.reshape(-1, P).T.copy(),
        ))

    return meta, per_core, dinv


# --------------------------------------------------------------------------
# Bass program
# --------------------------------------------------------------------------

def build_program(meta, c_in, c_hid, c_out, bn_eps=1e-5):
    n, T, ncp, R, Rh = meta["n"], meta["T"], meta["ncp"], meta["R"], meta["Rh"]
    n_per = meta["n_per"]
    NB_lo, NB_hi = meta["NB_lo"], meta["NB_hi"]
    groups = meta["groups"]
    tile_blocks = meta["tile_blocks"]
    assert c_in == P and c_hid == P

    f32 = mybir.dt.float32
    bf16 = mybir.dt.bfloat16
    i16 = mybir.dt.int16

    nc = bacc.Bacc(None, target_bir_lowering=False, debug=False, num_devices=M,
                   num_swdge_queues=4)

    # ---- I/O ----
    xT_d = nc.declare_dram_parameter("xT", [c_in, ncp], bf16, isOutput=False)
    W1_d = nc.declare_dram_parameter("W1b", [c_in, c_hid], bf16, isOutput=False)
    W2_d = nc.declare_dram_parameter("W2b", [c_hid, c_out], bf16, isOutput=False)
    dinv_d = nc.declare_dram_parameter("dinv_t", [P, T], f32, isOutput=False)
    gamma_d = nc.declare_dram_parameter("gamma_r", [1, c_hid], f32, isOutput=False)
    beta_d = nc.declare_dram_parameter("beta_r", [1, c_hid], f32, isOutput=False)
    b1_d = nc.declare_dram_parameter("b1_r", [1, c_hid], f32, isOutput=False)
    b2_d = nc.declare_dram_parameter("b2_r", [1, c_out], bf16, isOutput=False)
    iota_d = nc.declare_dram_parameter("iota_b", [P, P], bf16, isOutput=False)
    ident_d = nc.declare_dram_parameter("ident_b", [P, P], bf16, isOutput=False)
    onesc_d = nc.declare_dram_parameter("ones_col", [P, 1], bf16, isOutput=False)
    onesr_d = nc.declare_dram_parameter("ones_row", [1, P], bf16, isOutput=False)
    idxlo_d = nc.declare_dram_parameter("idx_lo", [P, max(NB_lo, 1) * 8], i16, isOutput=False)
    idxhi_d = nc.declare_dram_parameter("idx_hi", [P, max(NB_hi, 1) * 8], i16, isOutput=False)
    dllo_d = nc.declare_dram_parameter("dl_lo", [P, max(NB_lo, 1)], f32, isOutput=False)
    dlhi_d = nc.declare_dram_parameter("dl_hi", [P, max(NB_hi, 1)], f32, isOutput=False)
    zout_d = nc.declare_dram_parameter("zout", [n_per, c_out], f32, isOutput=True)

    # dma_gather's ucode lives in the gpsimd "mlp" library; load it before
    # any Tile-scheduled instructions run.
    nc.gpsimd.load_library(library_config.mlp)

    with tile.TileContext(nc) as tc:
        with (
            tc.tile_pool(name="dram", bufs=1, space="DRAM") as dram,
            tc.tile_pool(name="singles", bufs=1) as singles,
            tc.tile_pool(name="gather", bufs=4) as gpool,
            tc.tile_pool(name="sel", bufs=4) as spool,
            tc.tile_pool(name="scratch", bufs=3) as scratch,
            tc.tile_pool(name="psum", bufs=2, space="PSUM") as psum,
            tc.tile_pool(name="psumaux", bufs=2, space="PSUM") as psumaux,
            tc.tile_pool(name="psum1", bufs=1, space="PSUM") as psum1,
        ):
            # ---- persistent SBUF ----
            xT_s = singles.tile([c_in, ncp], bf16)
            W1_s = singles.tile([c_in, c_hid], bf16)
            W2_s = singles.tile([c_hid, c_out], bf16)
            dinv_s = singles.tile([P, T], f32)
            iota_s = singles.tile([P, P], bf16)
            ident_s = singles.tile([P, P], bf16)
            onesc_s = singles.tile([P, 1], bf16)
            onesr_s = singles.tile([1, P], bf16)
            gamma_s = singles.tile([1, c_hid], f32)
            beta_s = singles.tile([1, c_hid], f32)
            b1_s = singles.tile([1, c_hid], f32)
            b2r_s = singles.tile([1, c_out], bf16)
            idxlo_s = singles.tile([P, max(NB_lo, 1) * 8], i16)
            idxhi_s = singles.tile([P, max(NB_hi, 1) * 8], i16)
            dllo_s = singles.tile([P, max(NB_lo, 1)], f32)
            dlhi_s = singles.tile([P, max(NB_hi, 1)], f32)
            gbig_s = singles.tile([P, T, c_hid], bf16)   # AG staging / self rows
            Y_s = singles.tile([P, T, c_hid], bf16)      # BN input (dinv*agg1)
            zbig_s = singles.tile([P, T, c_out], f32)
            scsh_s = singles.tile([P, 2 * c_hid], bf16)  # BN scale/shift bcast
            b2c_s = singles.tile([P, c_out], f32)
            srow_s = singles.tile([1, 2 * c_hid], f32)   # local stat sums
            arres_s = singles.tile([1, 2 * c_hid], f32)  # allreduced sums
            rows_s = singles.tile([1, 8 * c_hid], f32)   # small row scratch
            scshrow_s = singles.tile([1, 2 * c_hid], bf16)

            # ---- internal DRAM (collective bounce) ----
            ag_in1 = dram.tile([ncp, c_hid], bf16)
            ag_out1 = dram.tile([R, c_hid], bf16, addr_space="Shared")
            ag_in2 = dram.tile([ncp, c_hid], bf16)
            ag_out2 = dram.tile([R, c_hid], bf16, addr_space="Shared")
            ar_in = dram.tile([1, 2 * c_hid], f32)
            ar_out = dram.tile([1, 2 * c_hid], f32, addr_space="Shared")

            # ---- load inputs ----
            for dst_t, src_t in ((xT_s, xT_d), (W1_s, W1_d), (W2_s, W2_d),
                                 (dinv_s, dinv_d), (iota_s, iota_d),
                                 (ident_s, ident_d), (onesc_s, onesc_d),
                                 (onesr_s, onesr_d), (gamma_s, gamma_d),
                                 (beta_s, beta_d), (b1_s, b1_d), (b2r_s, b2_d),
                                 (idxlo_s, idxlo_d), (idxhi_s, idxhi_d),
                                 (dllo_s, dllo_d), (dlhi_s, dlhi_d)):
                nc.sync.dma_start(out=dst_t[:], in_=src_t[:])

            # b2 broadcast tile [P, c_out]
            pb2 = psumaux.tile([P, 2 * c_hid], f32, tag="aux")
            nc.tensor.matmul(pb2[:, 0:c_out], lhsT=onesr_s[:], rhs=b2r_s[:],
                             start=True, stop=True)
            nc.vector.tensor_copy(b2c_s[:], pb2[:, 0:c_out])

            # ---- P1: g1 = (x @ W1) * dinv ----
            for t in range(T):
                ph = psum.tile([P, c_hid], f32)
                nc.tensor.matmul(ph[:], lhsT=xT_s[:, t * P:(t + 1) * P],
                                 rhs=W1_s[:], start=True, stop=True)
                nc.scalar.activation(gbig_s[:, t, :], ph[:], AF.Copy,
                                     scale=dinv_s[:, t:t + 1])
            nc.sync.dma_start(
                out=ag_in1[:].rearrange("(t p) h -> p t h", p=P),
                in_=gbig_s[:, :, :])

            # ---- P2: AllGather layer-1 table ----
            nc.gpsimd.collective_compute(
                "AllGather", AluOpType.bypass,
                replica_groups=[list(range(M))],
                ins=[ag_in1[:].opt()], outs=[ag_out1[:].opt()])

            ps1 = psum1.tile([1, c_hid], f32)
            ps2 = psum1.tile([1, c_hid], f32)

            # ---- aggregation pass (used for both layers) ----
            # One dma_gather's descriptors must fit the SWDGE ring carveout
            # (~num_idxs/16+1 descs per engine ring), so chunk large gathers.
            # Q7 descriptor generation (~8.4ns/idx per CPU pair) is the kernel
            # bottleneck; queue_num q dispatches to Q7 pair q, so round-robin
            # over the 4 SWDGE queues to generate descriptors 4-wide.
            GCAP = 8  # blocks per dma_gather (1024 idxs; ring fits ~65 descs/lane)
            qrr = [0]

            def chunked_gather(dst_tile, table_ap, idx_s, off, nb):
                for c0 in range(0, nb, GCAP):
                    sz = min(GCAP, nb - c0)
                    nc.gpsimd.dma_gather(
                        dst_tile[:, c0:c0 + sz, :], table_ap,
                        idx_s[:, (off + c0) * 8:(off + c0 + sz) * 8],
                        num_idxs=sz * P, num_idxs_reg=sz * P,
                        elem_size=c_hid, queue_num=qrr[0],
                        single_packet=False)
                    qrr[0] = (qrr[0] + 1) % 4

            def build_sel(dst_tile, dl_s_, off, nb):
                # One-hot selection matrices for nb blocks in one DVE op:
                # S[p, b, j] = (iota[p, j] == dl[p, off+b]).
                io = iota_s[:].unsqueeze(1).broadcast_to([P, nb, P])
                dl = dl_s_[:, off:off + nb].unsqueeze(2).broadcast_to([P, nb, P])
                nc.vector.tensor_tensor(out=dst_tile[:], in0=io, in1=dl,
                                        op=AluOpType.is_equal)

            def aggregation(ag_out, epilogue):
                for g in groups:
                    nb_lo, nb_hi = g["nb_lo"], g["nb_hi"]
                    glo = ghi = slo = shi = None
                    if nb_lo:
                        glo = gpool.tile([P, nb_lo, c_hid], bf16, tag="glo")
                        chunked_gather(glo, ag_out[0:Rh, :], idxlo_s,
                                       g["off_lo"], nb_lo)
                        slo = spool.tile([P, nb_lo, P], bf16, tag="slo")
                        build_sel(slo, dllo_s, g["off_lo"], nb_lo)
                    if nb_hi:
                        ghi = gpool.tile([P, nb_hi, c_hid], bf16, tag="ghi")
                        chunked_gather(ghi, ag_out[Rh:R, :], idxhi_s,
                                       g["off_hi"], nb_hi)
                        shi = spool.tile([P, nb_hi, P], bf16, tag="shi")
                        build_sel(shi, dlhi_s, g["off_hi"], nb_hi)
                    for t in g["tiles"]:
                        pu = psum.tile([P, c_hid], f32, tag="pu")
                        first = True
                        for half, gt, st, off in (
                                (0, glo, slo, g["off_lo"]),
                                (1, ghi, shi, g["off_hi"])):
                            for b in tile_blocks[t][half]:
                                nc.tensor.matmul(
                                    pu[:], lhsT=st[:, b - off, :],
                                    rhs=gt[:, b - off, :],
                                    start=first, stop=False)
                                first = False
                        # self-loop: += g_local[t]
                        nc.tensor.matmul(pu[:], lhsT=ident_s[:],
                                         rhs=gbig_s[:, t, :],
                                         start=first, stop=True)
                        epilogue(t, pu)

            # ---- L1 epilogue: y0 = dinv*u, stats ----
            def epi1(t, pu):
                nc.vector.tensor_scalar(
                    out=Y_s[:, t, :], in0=pu[:],
                    scalar1=dinv_s[:, t:t + 1], scalar2=None,
                    op0=AluOpType.mult)
                sq = scratch.tile([P, c_hid], bf16, tag="sq")
                nc.vector.tensor_tensor(out=sq[:], in0=Y_s[:, t, :],
                                        in1=Y_s[:, t, :], op=AluOpType.mult)
                nc.tensor.matmul(ps1[:], lhsT=onesc_s[:], rhs=Y_s[:, t, :],
                                 start=(t == 0), stop=(t == T - 1),
                                 skip_group_check=True)
                nc.tensor.matmul(ps2[:], lhsT=onesc_s[:], rhs=sq[:],
                                 start=(t == 0), stop=(t == T - 1),
                                 skip_group_check=True)

            aggregation(ag_out1, epi1)

            # ---- P4: BN statistics -> scale/shift ----
            nc.vector.tensor_copy(srow_s[:, 0:c_hid], ps1[:])
            nc.vector.tensor_copy(srow_s[:, c_hid:], ps2[:])
            nc.sync.dma_start(out=ar_in[:], in_=srow_s[:])
            nc.gpsimd.collective_compute(
                "AllReduce", AluOpType.add,
                replica_groups=[list(range(M))],
                ins=[ar_in[:].opt()], outs=[ar_out[:].opt()])
            nc.sync.dma_start(out=arres_s[:], in_=ar_out[:])

            H = c_hid
            mean_r = rows_s[:, 0:H]
            e2_r = rows_s[:, H:2 * H]
            var_r = rows_s[:, 2 * H:3 * H]
            rstd_r = rows_s[:, 3 * H:4 * H]
            scale_r = rows_s[:, 4 * H:5 * H]
            tmp_r = rows_s[:, 5 * H:6 * H]
            shift_r = rows_s[:, 6 * H:7 * H]
            nc.vector.tensor_scalar(out=mean_r, in0=arres_s[:, 0:H],
                                    scalar1=1.0 / n, scalar2=None,
                                    op0=AluOpType.mult)
            nc.vector.tensor_scalar(out=e2_r, in0=arres_s[:, H:],
                                    scalar1=1.0 / n, scalar2=None,
                                    op0=AluOpType.mult)
            nc.vector.tensor_tensor(out=var_r, in0=mean_r, in1=mean_r,
                                    op=AluOpType.mult)
            nc.vector.tensor_tensor(out=var_r, in0=e2_r, in1=var_r,
                                    op=AluOpType.subtract)
            nc.vector.tensor_scalar(out=var_r, in0=var_r, scalar1=float(bn_eps),
                                    scalar2=None, op0=AluOpType.add)
            nc.scalar.activation(rstd_r, var_r, AF.Sqrt)
            nc.vector.reciprocal(rstd_r, rstd_r)
            nc.vector.tensor_tensor(out=scale_r, in0=gamma_s[:], in1=rstd_r,
                                    op=AluOpType.mult)
            # bias-before-BN cancels in (y - mean): shift = beta - mean*scale
            nc.vector.tensor_tensor(out=tmp_r, in0=mean_r, in1=scale_r,
                                    op=AluOpType.mult)
            nc.vector.tensor_tensor(out=shift_r, in0=beta_s[:], in1=tmp_r,
                                    op=AluOpType.subtract)
            nc.vector.tensor_copy(scshrow_s[:, 0:H], scale_r)
            nc.vector.tensor_copy(scshrow_s[:, H:], shift_r)
            pbb = psumaux.tile([P, 2 * c_hid], f32, tag="aux")
            nc.tensor.matmul(pbb[:], lhsT=onesr_s[:], rhs=scshrow_s[:],
                             start=True, stop=True)
            nc.scalar.activation(scsh_s[:], pbb[:], AF.Copy)

            # ---- P5: BN apply + ELU + dinv prescale -> g2 (fused big ops) ----
            scale_b = scsh_s[:, 0:H].unsqueeze(1).broadcast_to([P, T, H])
            shift_b = scsh_s[:, H:].unsqueeze(1).broadcast_to([P, T, H])
            dinv_b = dinv_s[:].unsqueeze(2).broadcast_to([P, T, H])
            zz = singles.tile([P, T, c_hid], bf16)
            tmp = singles.tile([P, T, c_hid], bf16)
            nc.vector.tensor_tensor(out=zz[:], in0=Y_s[:, :, :],
                                    in1=scale_b, op=AluOpType.mult)
            nc.vector.tensor_tensor(out=zz[:], in0=zz[:],
                                    in1=shift_b, op=AluOpType.add)
            nc.scalar.activation(tmp[:], zz[:], AF.Relu, scale=-1.0)
            nc.scalar.activation(tmp[:], tmp[:], AF.Exp, scale=-1.0)
            # zz <- max(zz,0) - 1 ; then zz <- exp(-relu(-z)) + (max(z,0)-1)
            nc.vector.tensor_scalar(out=zz[:], in0=zz[:], scalar1=0.0,
                                    scalar2=-1.0, op0=AluOpType.max,
                                    op1=AluOpType.add)
            nc.vector.tensor_tensor(out=zz[:], in0=tmp[:], in1=zz[:],
                                    op=AluOpType.add)
            nc.vector.tensor_tensor(out=gbig_s[:, :, :], in0=zz[:],
                                    in1=dinv_b, op=AluOpType.mult)
            nc.sync.dma_start(
                out=ag_in2[:].rearrange("(t p) h -> p t h", p=P),
                in_=gbig_s[:, :, :])

            nc.gpsimd.collective_compute(
                "AllGather", AluOpType.bypass,
                replica_groups=[list(range(M))],
                ins=[ag_in2[:].opt()], outs=[ag_out2[:].opt()])

            # ---- L2 epilogue: z = (dinv*u) @ W2 + b2 ----
            def epi2(t, pu):
                A = scratch.tile([P, c_hid], bf16, tag="A")
                nc.scalar.activation(A[:], pu[:], AF.Copy,
                                     scale=dinv_s[:, t:t + 1])
                pt = psumaux.tile([P, 2 * c_hid], bf16, tag="aux")
                nc.tensor.transpose(out=pt[:, 0:P], in_=A[:], identity=ident_s[:])
                AT = scratch.tile([P, P], bf16, tag="AT")
                nc.vector.tensor_copy(AT[:], pt[:, 0:P])
                pz = psumaux.tile([P, 2 * c_hid], f32, tag="aux")
                nc.tensor.matmul(pz[:, 0:c_out], lhsT=AT[:], rhs=W2_s[:],
                                 start=True, stop=True)
                nc.vector.tensor_tensor(out=zbig_s[:, t, :], in0=pz[:, 0:c_out],
                                        in1=b2c_s[:], op=AluOpType.add)

            aggregation(ag_out2, epi2)

            # ---- output ----
            ft = n_per // P
            rem = n_per % P
            if ft:
                nc.sync.dma_start(
                    out=zout_d[0:ft * P, :].rearrange("(t p) f -> p t f", p=P),
                    in_=zbig_s[:, 0:ft, :])
            if rem:
                nc.sync.dma_start(
                    out=zout_d[ft * P:n_per, :],
                    in_=zbig_s[0:rem, ft, :])

    nc.compile()
    return nc


# --------------------------------------------------------------------------
# Input assembly
# --------------------------------------------------------------------------

def make_in_maps(x, W1, b1, gamma, beta, W2, b2, meta, per_core, dinv):
    n_per, T, ncp = meta["n_per"], meta["T"], meta["ncp"]
    c_in = x.shape[1]
    c_hid = W1.shape[1]
    c_out = W2.shape[1]

    iota = np.broadcast_to(np.arange(P, dtype=np.float32), (P, P)).astype(BF16)
    ident = np.eye(P, dtype=np.float32).astype(BF16)
    ones_col = np.ones((P, 1), dtype=np.float32).astype(BF16)
    ones_row = np.ones((1, P), dtype=np.float32).astype(BF16)
    W1b = np.asarray(W1, np.float32).astype(BF16)
    W2b = np.asarray(W2, np.float32).astype(BF16)

    in_maps = []
    for c in range(M):
        lo, hi = c * n_per, (c + 1) * n_per
        xTc = np.zeros((c_in, ncp), dtype=np.float32)
        xTc[:, :n_per] = np.asarray(x[lo:hi], np.float32).T
        dinv_c = np.zeros(ncp, dtype=np.float32)
        dinv_c[:n_per] = dinv[lo:hi]
        pc = per_core[c]
        in_maps.append({
            "xT": xTc.astype(BF16),
            "W1b": W1b, "W2b": W2b,
            "dinv_t": dinv_c.reshape(T, P).T.copy(),
            "gamma_r": np.asarray(gamma, np.float32).reshape(1, c_hid),
            "beta_r": np.asarray(beta, np.float32).reshape(1, c_hid),
            "b1_r": np.asarray(b1, np.float32).reshape(1, c_hid),
            "b2_r": np.asarray(b2, np.float32).reshape(1, c_out).astype(BF16),
            "iota_b": iota, "ident_b": ident,
            "ones_col": ones_col, "ones_row": ones_row,
            "idx_lo": pc["idx_lo"], "idx_hi": pc["idx_hi"],
            "dl_lo": pc["dl_lo"], "dl_hi": pc["dl_hi"],
        })
    return in_maps


# --------------------------------------------------------------------------
# Entry point
# --------------------------------------------------------------------------

_CACHE = {}


def _get_compiled(edge_index, n, c_in, c_hid, c_out):
    key = (n, c_in, c_hid, c_out,
           hash(np.asarray(edge_index).tobytes()))
    if key not in _CACHE:
        meta, per_core, dinv = preprocess(edge_index, n)
        nc = build_program(meta, c_in, c_hid, c_out)
        _CACHE[key] = (nc, meta, per_core, dinv)
    return _CACHE[key]


def kernel(x, edge_index, W1, b1, gamma, beta, W2, b2, _trace=False):
    x = np.asarray(x)
    n = x.shape[0]
    nc, meta, per_core, dinv = _get_compiled(
        edge_index, n, x.shape[1], W1.shape[1], W2.shape[1])
    in_maps = make_in_maps(x, W1, b1, gamma, beta, W2, b2,
                           meta, per_core, dinv)
    res = run_bass_kernel_spmd(nc, in_maps, core_ids=list(range(M)),
                               trace=_trace)
    outs = res.results
    z = np.concatenate([outs[c]["zout"] for c in range(M)], axis=0)
    kernel.last_result = res
    return z.astype(np.float32)



# revision 18
# speedup vs baseline: 1.0537x; 1.0537x over previous
"""Two-layer GCN encoder (GCNConv -> BatchNorm -> ELU -> GCNConv) on 8 trn2
NeuronCores.

Sharding: nodes are partitioned across the 8 cores (graph/data parallel).
Each core computes h = x_local @ W1 (pre-scaled by deg^-1/2), the scaled
feature table is AllGathered, and each core aggregates messages for its own
destination nodes by gathering source rows with dma_gather and scatter-adding
them into PSUM via one-hot selection-matrix matmuls (128 edges per matmul).
BN statistics are AllReduced. Layer 2 aggregates the 128-wide hidden features
first and applies W2 afterwards ( A(hW) == (Ah)W ).

Self-loops are applied as an extra identity matmul per destination tile (the
local g rows are resident in SBUF), not as explicit edges.

Performance notes (measured on trn2):
- The kernel is bound by SWDGE Q7 descriptor generation for the per-edge
  dma_gathers (~8.4 ns/index per Q7 pair) plus the 256B-granular random-read
  drain (~7 GB/s per SDMA engine).  dma_gather's ucode dispatches to Q7 CPU
  pair `queue_num`, so gathers are round-robined over all 4 SWDGE queues
  (num_swdge_queues=4) to generate descriptors on all 8 Q7 cores (~2.4x).
- One-hot selection matrices are built in bulk (one broadcast-AP
  tensor_tensor per tile-group-half) instead of per 128-edge block.
- BN apply + ELU + dinv prescale runs as a handful of whole-table fused ops.
- GCAP=8 (1024 idxs/call) is the SWDGE ring capacity limit; larger values
  hang the ucode's ring-space wait.  single_packet=False is a regression.
"""

import numpy as np
import ml_dtypes

import concourse.bass as bass
import concourse.bacc as bacc
import concourse.mybir as mybir
import concourse.tile as tile
from concourse.alu_op_type import AluOpType
from concourse import library_config
from concourse.bass_utils import run_bass_kernel_spmd

P = 128
M = 8  # cores
BF16 = ml_dtypes.bfloat16
AF = mybir.ActivationFunctionType


# --------------------------------------------------------------------------
# Host-side preprocessing
# --------------------------------------------------------------------------

def preprocess(edge_index, n, gt_tiles=2):
    """Sort/partition edges, build per-core padded index arrays and the
    (uniform across cores) block structure.

    Returns (meta, per_core) where meta has the compile-time structure and
    per_core the per-core numpy arrays.
    """
    src = np.asarray(edge_index[0], dtype=np.int64)
    dst = np.asarray(edge_index[1], dtype=np.int64)
    e = src.shape[0]

    n_per = n // M
    assert n_per * M == n
    T = (n_per + P - 1) // P
    ncp = T * P                      # padded nodes per core
    R = M * ncp                      # rows in the gathered table
    Rh = R // 2                      # half split (must fit int16)
    assert Rh < 32768, f"half table {Rh} rows exceeds int16 range"
    assert (4 * ncp) == Rh           # half boundary aligns with core boundary

    deg = np.bincount(dst, minlength=n).astype(np.float64) + 1.0
    dinv = (deg ** -0.5).astype(np.float32)

    # padded global row of each source node
    owner = src // n_per
    r_src = owner * ncp + (src - owner * n_per)          # [e]
    half = (r_src >= Rh).astype(np.int64)
    idx16 = (r_src - half * Rh).astype(np.int64)

    core_of = dst // n_per
    l_dst = dst - core_of * n_per
    t_dst = l_dst // P
    dstloc = l_dst % P

    # per (core, tile, half) edge counts -> uniform block counts
    counts = np.zeros((M, T, 2), dtype=np.int64)
    np.add.at(counts, (core_of, t_dst, half), 1)
    Bth = np.ceil(counts.max(axis=0) / P).astype(np.int64)   # [T, 2]

    # groups of tiles for gathers
    groups = []
    off_lo = off_hi = 0
    for g0 in range(0, T, gt_tiles):
        tiles = list(range(g0, min(g0 + gt_tiles, T)))
        nb_lo = int(sum(Bth[t, 0] for t in tiles))
        nb_hi = int(sum(Bth[t, 1] for t in tiles))
        groups.append(dict(tiles=tiles, off_lo=off_lo, nb_lo=nb_lo,
                           off_hi=off_hi, nb_hi=nb_hi))
        off_lo += nb_lo
        off_hi += nb_hi
    NB_lo, NB_hi = off_lo, off_hi

    # per-tile block index lists (global block number within its half array)
    tile_blocks = []   # [T] -> (lo_block_ids, hi_block_ids)
    blo = bhi = 0
    for t in range(T):
        lo_ids = list(range(blo, blo + int(Bth[t, 0]))); blo += int(Bth[t, 0])
        hi_ids = list(range(bhi, bhi + int(Bth[t, 1]))); bhi += int(Bth[t, 1])
        tile_blocks.append((lo_ids, hi_ids))

    meta = dict(n=n, e=e, n_per=n_per, T=T, ncp=ncp, R=R, Rh=Rh,
                Bth=Bth, groups=groups, tile_blocks=tile_blocks,
                NB_lo=NB_lo, NB_hi=NB_hi)

    # --- per-core arrays ---
    order = np.lexsort((half, t_dst, core_of))   # sort by core, tile, half
    src_s = idx16[order]
    dl_s = dstloc[order]
    c_s = core_of[order]
    t_s = t_dst[order]
    h_s = half[order]

    per_core = []
    # compute start offset of each (c,t,h) run via counts prefix
    flat_counts = counts  # [M,T,2]
    run_start = np.zeros((M, T, 2), dtype=np.int64)
    np.cumsum(flat_counts.reshape(-1), out=run_start.reshape(-1))
    run_start = run_start - flat_counts  # exclusive prefix

    for c in range(M):
        idx_lo = np.zeros(max(NB_lo, 1) * P, dtype=np.int16)
        dl_lo = np.full(max(NB_lo, 1) * P, 255.0, dtype=np.float32)
        idx_hi = np.zeros(max(NB_hi, 1) * P, dtype=np.int16)
        dl_hi = np.full(max(NB_hi, 1) * P, 255.0, dtype=np.float32)
        for t in range(T):
            for h, (idx_arr, dl_arr, blk_ids) in enumerate(
                    ((idx_lo, dl_lo, tile_blocks[t][0]),
                     (idx_hi, dl_hi, tile_blocks[t][1]))):
                cnt = int(flat_counts[c, t, h])
                s0 = int(run_start[c, t, h])
                if not blk_ids:
                    assert cnt == 0
                    continue
                dst_off = blk_ids[0] * P
                idx_arr[dst_off:dst_off + cnt] = src_s[s0:s0 + cnt]
                dl_arr[dst_off:dst_off + cnt] = dl_s[s0:s0 + cnt]
                assert cnt <= len(blk_ids) * P

        def wrap16(a):
            w = a.reshape(-1, 16).T.copy()          # [16, L/16]
            return np.tile(w, (8, 1))               # [128, L/16]

        per_core.append(dict(
            idx_lo=wrap16(idx_lo), idx_hi=wrap16(idx_hi),
            dl_lo=dl_lo.reshape(-1, P).T.copy(),
            dl_hi=dl_hi.reshape(-1, P).T.copy(),
        ))

    return meta, per_core, dinv


# --------------------------------------------------------------------------
# Bass program
# --------------------------------------------------------------------------

def build_program(meta, c_in, c_hid, c_out, bn_eps=1e-5):
    n, T, ncp, R, Rh = meta["n"], meta["T"], meta["ncp"], meta["R"], meta["Rh"]
    n_per = meta["n_per"]
    NB_lo, NB_hi = meta["NB_lo"], meta["NB_hi"]
    groups = meta["groups"]
    tile_blocks = meta["tile_blocks"]
    assert c_in == P and c_hid == P

    f32 = mybir.dt.float32
    bf16 = mybir.dt.bfloat16
    i16 = mybir.dt.int16

    nc = bacc.Bacc(None, target_bir_lowering=False, debug=False, num_devices=M,
                   num_swdge_queues=4)

    # ---- I/O ----
    xT_d = nc.declare_dram_parameter("xT", [c_in, ncp], bf16, isOutput=False)
    W1_d = nc.declare_dram_parameter("W1b", [c_in, c_hid], bf16, isOutput=False)
    W2_d = nc.declare_dram_parameter("W2b", [c_hid, c_out], bf16, isOutput=False)
    dinv_d = nc.declare_dram_parameter("dinv_t", [P, T], f32, isOutput=False)
    gamma_d = nc.declare_dram_parameter("gamma_r", [1, c_hid], f32, isOutput=False)
    beta_d = nc.declare_dram_parameter("beta_r", [1, c_hid], f32, isOutput=False)
    b1_d = nc.declare_dram_parameter("b1_r", [1, c_hid], f32, isOutput=False)
    b2_d = nc.declare_dram_parameter("b2_r", [1, c_out], bf16, isOutput=False)
    iota_d = nc.declare_dram_parameter("iota_b", [P, P], bf16, isOutput=False)
    ident_d = nc.declare_dram_parameter("ident_b", [P, P], bf16, isOutput=False)
    onesc_d = nc.declare_dram_parameter("ones_col", [P, 1], bf16, isOutput=False)
    onesr_d = nc.declare_dram_parameter("ones_row", [1, P], bf16, isOutput=False)
    idxlo_d = nc.declare_dram_parameter("idx_lo", [P, max(NB_lo, 1) * 8], i16, isOutput=False)
    idxhi_d = nc.declare_dram_parameter("idx_hi", [P, max(NB_hi, 1) * 8], i16, isOutput=False)
    dllo_d = nc.declare_dram_parameter("dl_lo", [P, max(NB_lo, 1)], f32, isOutput=False)
    dlhi_d = nc.declare_dram_parameter("dl_hi", [P, max(NB_hi, 1)], f32, isOutput=False)
    zout_d = nc.declare_dram_parameter("zout", [n_per, c_out], f32, isOutput=True)

    # dma_gather's ucode lives in the gpsimd "mlp" library; load it before
    # any Tile-scheduled instructions run.
    nc.gpsimd.load_library(library_config.mlp)

    with tile.TileContext(nc) as tc:
        with (
            tc.tile_pool(name="dram", bufs=1, space="DRAM") as dram,
            tc.tile_pool(name="singles", bufs=1) as singles,
            tc.tile_pool(name="gather", bufs=4) as gpool,
            tc.tile_pool(name="sel", bufs=4) as spool,
            tc.tile_pool(name="scratch", bufs=3) as scratch,
            tc.tile_pool(name="psum", bufs=2, space="PSUM") as psum,
            tc.tile_pool(name="psumaux", bufs=2, space="PSUM") as psumaux,
            tc.tile_pool(name="psum1", bufs=1, space="PSUM") as psum1,
        ):
            # ---- persistent SBUF ----
            xT_s = singles.tile([c_in, ncp], bf16)
            W1_s = singles.tile([c_in, c_hid], bf16)
            W2_s = singles.tile([c_hid, c_out], bf16)
            dinv_s = singles.tile([P, T], f32)
            iota_s = singles.tile([P, P], bf16)
            ident_s = singles.tile([P, P], bf16)
            onesc_s = singles.tile([P, 1], bf16)
            onesr_s = singles.tile([1, P], bf16)
            gamma_s = singles.tile([1, c_hid], f32)
            beta_s = singles.tile([1, c_hid], f32)
            b1_s = singles.tile([1, c_hid], f32)
            b2r_s = singles.tile([1, c_out], bf16)
            idxlo_s = singles.tile([P, max(NB_lo, 1) * 8], i16)
            idxhi_s = singles.tile([P, max(NB_hi, 1) * 8], i16)
            dllo_s = singles.tile([P, max(NB_lo, 1)], f32)
            dlhi_s = singles.tile([P, max(NB_hi, 1)], f32)
            gbig_s = singles.tile([P, T, c_hid], bf16)   # AG staging / self rows
            Y_s = singles.tile([P, T, c_hid], bf16)      # BN input (dinv*agg1)
            zbig_s = singles.tile([P, T, c_out], f32)
            scsh_s = singles.tile([P, 2 * c_hid], bf16)  # BN scale/shift bcast
            b2c_s = singles.tile([P, c_out], f32)
            srow_s = singles.tile([1, 2 * c_hid], f32)   # local stat sums
            arres_s = singles.tile([1, 2 * c_hid], f32)  # allreduced sums
            rows_s = singles.tile([1, 8 * c_hid], f32)   # small row scratch
            scshrow_s = singles.tile([1, 2 * c_hid], bf16)

            # ---- internal DRAM (collective bounce) ----
            ag_in1 = dram.tile([ncp, c_hid], bf16)
            ag_out1 = dram.tile([R, c_hid], bf16, addr_space="Shared")
            ag_in2 = dram.tile([ncp, c_hid], bf16)
            ag_out2 = dram.tile([R, c_hid], bf16, addr_space="Shared")
            ar_in = dram.tile([1, 2 * c_hid], f32)
            ar_out = dram.tile([1, 2 * c_hid], f32, addr_space="Shared")

            # ---- load inputs ----
            for dst_t, src_t in ((xT_s, xT_d), (W1_s, W1_d), (W2_s, W2_d),
                                 (dinv_s, dinv_d), (iota_s, iota_d),
                                 (ident_s, ident_d), (onesc_s, onesc_d),
                                 (onesr_s, onesr_d), (gamma_s, gamma_d),
                                 (beta_s, beta_d), (b1_s, b1_d), (b2r_s, b2_d),
                                 (idxlo_s, idxlo_d), (idxhi_s, idxhi_d),
                                 (dllo_s, dllo_d), (dlhi_s, dlhi_d)):
                nc.sync.dma_start(out=dst_t[:], in_=src_t[:])

            # b2 broadcast tile [P, c_out]
            pb2 = psumaux.tile([P, 2 * c_hid], f32, tag="aux")
            nc.tensor.matmul(pb2[:, 0:c_out], lhsT=onesr_s[:], rhs=b2r_s[:],
                             start=True, stop=True)
            nc.vector.tensor_copy(b2c_s[:], pb2[:, 0:c_out])

            # ---- P1: g1 = (x @ W1) * dinv ----
            for t in range(T):
                ph = psum.tile([P, c_hid], f32)
                nc.tensor.matmul(ph[:], lhsT=xT_s[:, t * P:(t + 1) * P],
                                 rhs=W1_s[:], start=True, stop=True)
                nc.scalar.activation(gbig_s[:, t, :], ph[:], AF.Copy,
                                     scale=dinv_s[:, t:t + 1])
            nc.sync.dma_start(
                out=ag_in1[:].rearrange("(t p) h -> p t h", p=P),
                in_=gbig_s[:, :, :])

            # ---- P2: AllGather layer-1 table ----
            nc.gpsimd.collective_compute(
                "AllGather", AluOpType.bypass,
                replica_groups=[list(range(M))],
                ins=[ag_in1[:].opt()], outs=[ag_out1[:].opt()])

            ps1 = psum1.tile([1, c_hid], f32)
            ps2 = psum1.tile([1, c_hid], f32)

            # ---- aggregation pass (used for both layers) ----
            # One dma_gather's descriptors must fit the SWDGE ring carveout
            # (~num_idxs/16+1 descs per engine ring), so chunk large gathers.
            # Q7 descriptor generation (~8.4ns/idx per CPU pair) is the kernel
            # bottleneck; queue_num q dispatches to Q7 pair q, so round-robin
            # over the 4 SWDGE queues to generate descriptors 4-wide.
            GCAP = 8  # blocks per dma_gather (1024 idxs; ring fits ~65 descs/lane)
            qrr = [0]

            def chunked_gather(dst_tile, table_ap, idx_s, off, nb):
                for c0 in range(0, nb, GCAP):
                    sz = min(GCAP, nb - c0)
                    nc.gpsimd.dma_gather(
                        dst_tile[:, c0:c0 + sz, :], table_ap,
                        idx_s[:, (off + c0) * 8:(off + c0 + sz) * 8],
                        num_idxs=sz * P, num_idxs_reg=sz * P,
                        elem_size=c_hid, queue_num=qrr[0])
                    qrr[0] = (qrr[0] + 1) % 4

            def build_sel(dst_tile, dl_s_, off, nb):
                # One-hot selection matrices for nb blocks in one DVE op:
                # S[p, b, j] = (iota[p, j] == dl[p, off+b]).
                io = iota_s[:].unsqueeze(1).broadcast_to([P, nb, P])
                dl = dl_s_[:, off:off + nb].unsqueeze(2).broadcast_to([P, nb, P])
                nc.vector.tensor_tensor(out=dst_tile[:], in0=io, in1=dl,
                                        op=AluOpType.is_equal)

            def aggregation(ag_out, epilogue):
                for g in groups:
                    nb_lo, nb_hi = g["nb_lo"], g["nb_hi"]
                    glo = ghi = slo = shi = None
                    if nb_lo:
                        glo = gpool.tile([P, nb_lo, c_hid], bf16, tag="glo")
                        chunked_gather(glo, ag_out[0:Rh, :], idxlo_s,
                                       g["off_lo"], nb_lo)
                        slo = spool.tile([P, nb_lo, P], bf16, tag="slo")
                        build_sel(slo, dllo_s, g["off_lo"], nb_lo)
                    if nb_hi:
                        ghi = gpool.tile([P, nb_hi, c_hid], bf16, tag="ghi")
                        chunked_gather(ghi, ag_out[Rh:R, :], idxhi_s,
                                       g["off_hi"], nb_hi)
                        shi = spool.tile([P, nb_hi, P], bf16, tag="shi")
                        build_sel(shi, dlhi_s, g["off_hi"], nb_hi)
                    for t in g["tiles"]:
                        pu = psum.tile([P, c_hid], f32, tag="pu")
                        first = True
                        for half, gt, st, off in (
                                (0, glo, slo, g["off_lo"]),
                                (1, ghi, shi, g["off_hi"])):
                            for b in tile_blocks[t][half]:
                                nc.tensor.matmul(
                                    pu[:], lhsT=st[:, b - off, :],
                                    rhs=gt[:, b - off, :],
                                    start=first, stop=False)
                                first = False
                        # self-loop: += g_local[t]
                        nc.tensor.matmul(pu[:], lhsT=ident_s[:],
                                         rhs=gbig_s[:, t, :],
                                         start=first, stop=True)
                        epilogue(t, pu)

            # ---- L1 epilogue: y0 = dinv*u, stats ----
            def epi1(t, pu):
                nc.vector.tensor_scalar(
                    out=Y_s[:, t, :], in0=pu[:],
                    scalar1=dinv_s[:, t:t + 1], scalar2=None,
                    op0=AluOpType.mult)
                sq = scratch.tile([P, c_hid], bf16, tag="sq")
                nc.vector.tensor_tensor(out=sq[:], in0=Y_s[:, t, :],
                                        in1=Y_s[:, t, :], op=AluOpType.mult)
                nc.tensor.matmul(ps1[:], lhsT=onesc_s[:], rhs=Y_s[:, t, :],
                                 start=(t == 0), stop=(t == T - 1),
                                 skip_group_check=True)
                nc.tensor.matmul(ps2[:], lhsT=onesc_s[:], rhs=sq[:],
                                 start=(t == 0), stop=(t == T - 1),
                                 skip_group_check=True)

            aggregation(ag_out1, epi1)

            # ---- P4: BN statistics -> scale/shift ----
            nc.vector.tensor_copy(srow_s[:, 0:c_hid], ps1[:])
            nc.vector.tensor_copy(srow_s[:, c_hid:], ps2[:])
            nc.sync.dma_start(out=ar_in[:], in_=srow_s[:])
            nc.gpsimd.collective_compute(
                "AllReduce", AluOpType.add,
                replica_groups=[list(range(M))],
                ins=[ar_in[:].opt()], outs=[ar_out[:].opt()])
            nc.sync.dma_start(out=arres_s[:], in_=ar_out[:])

            H = c_hid
            mean_r = rows_s[:, 0:H]
            e2_r = rows_s[:, H:2 * H]
            var_r = rows_s[:, 2 * H:3 * H]
            rstd_r = rows_s[:, 3 * H:4 * H]
            scale_r = rows_s[:, 4 * H:5 * H]
            tmp_r = rows_s[:, 5 * H:6 * H]
            shift_r = rows_s[:, 6 * H:7 * H]
            nc.vector.tensor_scalar(out=mean_r, in0=arres_s[:, 0:H],
                                    scalar1=1.0 / n, scalar2=None,
                                    op0=AluOpType.mult)
            nc.vector.tensor_scalar(out=e2_r, in0=arres_s[:, H:],
                                    scalar1=1.0 / n, scalar2=None,
                                    op0=AluOpType.mult)
            nc.vector.tensor_tensor(out=var_r, in0=mean_r, in1=mean_r,
                                    op=AluOpType.mult)
            nc.vector.tensor_tensor(out=var_r, in0=e2_r, in1=var_r,
                                    op=AluOpType.subtract)
            nc.vector.tensor_scalar(out=var_r, in0=var_r, scalar1=float(bn_eps),
                                    scalar2=None, op0=AluOpType.add)
            nc.scalar.activation(rstd_r, var_r, AF.Sqrt)
            nc.vector.reciprocal(rstd_r, rstd_r)
            nc.vector.tensor_tensor(out=scale_r, in0=gamma_s[:], in1=rstd_r,
                                    op=AluOpType.mult)
            # bias-before-BN cancels in (y - mean): shift = beta - mean*scale
            nc.vector.tensor_tensor(out=tmp_r, in0=mean_r, in1=scale_r,
                                    op=AluOpType.mult)
            nc.vector.tensor_tensor(out=shift_r, in0=beta_s[:], in1=tmp_r,
                                    op=AluOpType.subtract)
            nc.vector.tensor_copy(scshrow_s[:, 0:H], scale_r)
            nc.vector.tensor_copy(scshrow_s[:, H:], shift_r)
            pbb = psumaux.tile([P, 2 * c_hid], f32, tag="aux")
            nc.tensor.matmul(pbb[:], lhsT=onesr_s[:], rhs=scshrow_s[:],
                             start=True, stop=True)
            nc.scalar.activation(scsh_s[:], pbb[:], AF.Copy)

            # ---- P5: BN apply + ELU + dinv prescale -> g2 (fused big ops) ----
            scale_b = scsh_s[:, 0:H].unsqueeze(1).broadcast_to([P, T, H])
            shift_b = scsh_s[:, H:].unsqueeze(1).broadcast_to([P, T, H])
            dinv_b = dinv_s[:].unsqueeze(2).broadcast_to([P, T, H])
            zz = singles.tile([P, T, c_hid], bf16)
            tmp = singles.tile([P, T, c_hid], bf16)
            nc.vector.tensor_tensor(out=zz[:], in0=Y_s[:, :, :],
                                    in1=scale_b, op=AluOpType.mult)
            nc.vector.tensor_tensor(out=zz[:], in0=zz[:],
                                    in1=shift_b, op=AluOpType.add)
            nc.scalar.activation(tmp[:], zz[:], AF.Relu, scale=-1.0)
            nc.scalar.activation(tmp[:], tmp[:], AF.Exp, scale=-1.0)
            # zz <- max(zz,0) - 1 ; then zz <- exp(-relu(-z)) + (max(z,0)-1)
            nc.vector.tensor_scalar(out=zz[:], in0=zz[:], scalar1=0.0,
                                    scalar2=-1.0, op0=AluOpType.max,
                                    op1=AluOpType.add)
            nc.vector.tensor_tensor(out=zz[:], in0=tmp[:], in1=zz[:],
                                    op=AluOpType.add)
            nc.vector.tensor_tensor(out=gbig_s[:, :, :], in0=zz[:],
                                    in1=dinv_b, op=AluOpType.mult)
            nc.sync.dma_start(
                out=ag_in2[:].rearrange("(t p) h -> p t h", p=P),
                in_=gbig_s[:, :, :])

            nc.gpsimd.collective_compute(
                "AllGather", AluOpType.bypass,
                replica_groups=[list(range(M))],
                ins=[ag_in2[:].opt()], outs=[ag_out2[:].opt()])

            # ---- L2 epilogue: z = (dinv*u) @ W2 + b2 ----
            def epi2(t, pu):
                A = scratch.tile([P, c_hid], bf16, tag="A")
                nc.scalar.activation(A[:], pu[:], AF.Copy,
                                     scale=dinv_s[:, t:t + 1])
                pt = psumaux.tile([P, 2 * c_hid], bf16, tag="aux")
                nc.tensor.transpose(out=pt[:, 0:P], in_=A[:], identity=ident_s[:])
                AT = scratch.tile([P, P], bf16, tag="AT")
                nc.vector.tensor_copy(AT[:], pt[:, 0:P])
                pz = psumaux.tile([P, 2 * c_hid], f32, tag="aux")
                nc.tensor.matmul(pz[:, 0:c_out], lhsT=AT[:], rhs=W2_s[:],
                                 start=True, stop=True)
                nc.vector.tensor_tensor(out=zbig_s[:, t, :], in0=pz[:, 0:c_out],
                                        in1=b2c_s[:], op=AluOpType.add)

            aggregation(ag_out2, epi2)

            # ---- output ----
            ft = n_per // P
            rem = n_per % P
            if ft:
                nc.sync.dma_start(
                    out=zout_d[0:ft * P, :].rearrange("(t p) f -> p t f", p=P),
                    in_=zbig_s[:, 0:ft, :])
            if rem:
                nc.sync.dma_start(
                    out=zout_d[ft * P:n_per, :],
                    in_=zbig_s[0:rem, ft, :])

    nc.compile()
    return nc


# --------------------------------------------------------------------------
# Input assembly
# --------------------------------------------------------------------------

def make_in_maps(x, W1, b1, gamma, beta, W2, b2, meta, per_core, dinv):
    n_per, T, ncp = meta["n_per"], meta["T"], meta["ncp"]
    c_in = x.shape[1]
    c_hid = W1.shape[1]
    c_out = W2.shape[1]

    iota = np.broadcast_to(np.arange(P, dtype=np.float32), (P, P)).astype(BF16)
    ident = np.eye(P, dtype=np.float32).astype(BF16)
    ones_col = np.ones((P, 1), dtype=np.float32).astype(BF16)
    ones_row = np.ones((1, P), dtype=np.float32).astype(BF16)
    W1b = np.asarray(W1, np.float32).astype(BF16)
    W2b = np.asarray(W2, np.float32).astype(BF16)

    in_maps = []
    for c in range(M):
        lo, hi = c * n_per, (c + 1) * n_per
        xTc = np.zeros((c_in, ncp), dtype=np.float32)
        xTc[:, :n_per] = np.asarray(x[lo:hi], np.float32).T
        dinv_c = np.zeros(ncp, dtype=np.float32)
        dinv_c[:n_per] = dinv[lo:hi]
        pc = per_core[c]
        in_maps.append({
            "xT": xTc.astype(BF16),
            "W1b": W1b, "W2b": W2b,
            "dinv_t": dinv_c.reshape(T, P).T.copy(),
            "gamma_r": np.asarray(gamma, np.float32).reshape(1, c_hid),
            "beta_r": np.asarray(beta, np.float32).reshape(1, c_hid),
            "b1_r": np.asarray(b1, np.float32).reshape(1, c_hid),
            "b2_r": np.asarray(b2, np.float32).reshape(1, c_out).astype(BF16),
            "iota_b": iota, "ident_b": ident,
            "ones_col": ones_col, "ones_row": ones_row,
            "idx_lo": pc["idx_lo"], "idx_hi": pc["idx_hi"],
            "dl_lo": pc["dl_lo"], "dl_hi": pc["dl_hi"],
        })
    return in_maps


# --------------------------------------------------------------------------
# Entry point
# --------------------------------------------------------------------------

_CACHE = {}


def _get_compiled(edge_index, n, c_in, c_hid, c_out):
    key = (n, c_in, c_hid, c_out,
           hash(np.asarray(edge_index).tobytes()))
    if key not in _CACHE:
        meta, per_core, dinv = preprocess(edge_index, n)
        nc = build_program(meta, c_in, c_hid, c_out)
        _CACHE[key] = (nc, meta, per_core, dinv)
    return _CACHE[key]


def kernel(x, edge_index, W1, b1, gamma, beta, W2, b2, _trace=False):
    x = np.asarray(x)
    n = x.shape[0]
    nc, meta, per_core, dinv = _get_compiled(
        edge_index, n, x.shape[1], W1.shape[1], W2.shape[1])
    in_maps = make_in_maps(x, W1, b1, gamma, beta, W2, b2,
                           meta, per_core, dinv)
    res = run_bass_kernel_spmd(nc, in_maps, core_ids=list(range(M)),
                               trace=_trace)
    outs = res.results
    z = np.concatenate([outs[c]["zout"] for c in range(M)], axis=0)
    kernel.last_result = res
    return z.astype(np.float32)

